# revision 4
# baseline (speedup 1.0000x reference)
"""MLA attention kernel for Trainium2 (8 NeuronCores, Bass/Tile).

Problem (nn_MLAAttention): B=2, S=2048, D=2048, 16 heads x 128, latent 512,
RoPE, causal softmax, output projection.

Sharding: core c handles batch b=c//4 and 4 heads hg=c%4 (tensor parallel over
heads, data parallel over batch). Each core computes a partial output
(attn-out of its heads through its Wo column-slice); the host sums partials
within a batch and transposes back.

On-core dataflow (all activations kept feature-major, "transposed"):
  XT[d,s] --Wq/Wc--> qT[dh,s] (+RoPE), c_kvT[dl,s]
  c_kvT --Wk--> kT[dh,s] (+RoPE);  c_kvT --Wv--> v[s,hf] (natural)
  scoresT[sk,sq] = kT.T-slice @ qT-slice  (per head, causal tiles only)
  expT = exp(scores/sqrt(dh)) * causal_mask
  sums[1,sq] = ones.T @ expT (partition reduce via PE)
  oT~[dh,sq] = v-slice.T @ expT  (unnormalized)
  oT = oT~ * bcast(1/sums)   (PE K=1 broadcast matmul + DVE reciprocal/mul)
  outT[dout,s] = WoT.T-slices @ oT  (partial output projection)

All matmuls run as float32r (full rate, ~1e-4 rounding vs fp32).
"""

import sys

if "/opt/trn_rl_repo" not in sys.path:
    sys.path.insert(0, "/opt/trn_rl_repo")

import math
import numpy as np

import concourse.bass as bass
import concourse.tile as tile
from concourse import bacc, mybir
from concourse.bass_utils import run_bass_kernel_spmd

F32 = mybir.dt.float32
F32R = mybir.dt.float32r
EXP = mybir.ActivationFunctionType.Exp

B, S, D, DL, H, DH = 2, 2048, 2048, 512, 16, 128
HL = 4  # heads per core
HF = HL * DH  # 512 local head-features
NCORES = 8
SCALE = 1.0 / math.sqrt(DH)

_CACHED = {}


def _build():
    nc = bacc.Bacc("TRN2", target_bir_lowering=False, debug=False)

    xt_d = nc.dram_tensor("xt", [D, S], F32R, kind="ExternalInput")
    wqct_d = nc.dram_tensor("wqct", [128, 16, 1024], F32R, kind="ExternalInput")
    wkvt_d = nc.dram_tensor("wkvt", [128, 4, 1024], F32R, kind="ExternalInput")
    wot_d = nc.dram_tensor("wot", [128, 4, 2048], F32R, kind="ExternalInput")
    cos_d = nc.dram_tensor("cost", [128, S], F32, kind="ExternalInput")
    sin_d = nc.dram_tensor("sint", [128, S], F32, kind="ExternalInput")
    mask_d = nc.dram_tensor("masks", [128, 4, 512], F32, kind="ExternalInput")
    ones_d = nc.dram_tensor("ones", [128, 128], F32R, kind="ExternalInput")
    out_d = nc.dram_tensor("outt", [D, S], F32, kind="ExternalOutput")

    with tile.TileContext(nc) as tc:
        _body(nc, tc, xt_d, wqct_d, wkvt_d, wot_d, cos_d, sin_d, mask_d, ones_d, out_d)
    nc.compile()
    return nc


def _rope_evac(nc, tmp_pool, psum, out_ap, cos_ap, sin_ap):
    """out = psum*cos + rotate_half(psum)*sin, written as float32r.

    psum: [128, n] PSUM (dh on partitions); cos/sin: [128, n] SBUF slices.
    """
    n = psum.shape[-1]
    t1 = tmp_pool.tile([128, 512], F32, tag="rope1")
    nc.vector.tensor_mul(t1[:, :n], psum[:], cos_ap)
    t2 = tmp_pool.tile([128, 512], F32, tag="rope2")
    nc.vector.tensor_mul(t2[0:64, :n], psum[64:128, :], sin_ap[0:64, :])
    nc.vector.tensor_mul(t2[64:128, :n], psum[0:64, :], sin_ap[64:128, :])
    nc.vector.tensor_sub(out_ap[0:64, :], t1[0:64, :n], t2[0:64, :n])
    nc.vector.tensor_add(out_ap[64:128, :], t1[64:128, :n], t2[64:128, :n])


def _body(nc, tc, xt_d, wqct_d, wkvt_d, wot_d, cos_d, sin_d, mask_d, ones_d, out_d):
    import contextlib

    with contextlib.ExitStack() as ctx:
        ent = ctx.enter_context

        # ---- persistent pools ----
        p_qt = ent(tc.tile_pool(name="qt", bufs=1))  # qT, later reused as oT
        p_kv = ent(tc.tile_pool(name="kv", bufs=1))  # kT + v
        p_small = ent(tc.tile_pool(name="small", bufs=1))  # ones etc

        qt = p_qt.tile([128, HL, S], F32R, tag="qt")  # [dh, h, s]
        kt = p_kv.tile([128, HL, S], F32R, tag="kt")  # [dh, h, s]
        v = p_kv.tile([128, 16, HF], F32R, tag="v")  # [s%128, s//128, hf]
        ones_col = p_small.tile([128, 1], F32R, tag="ones_col")
        ones_row = p_small.tile([1, 128], F32R, tag="ones_row")

        nc.sync.dma_start(ones_col[:], ones_d[:, 0:1])
        nc.sync.dma_start(ones_row[:], ones_d[0:1, :])

        # ============ Phases A+B: projections (scoped pools) ============
        with (
            tc.tile_pool(name="ck", bufs=1) as p_ck,
            tc.tile_pool(name="tmp", bufs=3) as p_tmp,
        ):
            ckv = p_ck.tile([128, 4, S], F32R, tag="ckv")  # [dl%128, dl//128, s]
            cost = p_ck.tile([128, S], F32, tag="cos")
            sint = p_ck.tile([128, S], F32, tag="sin")
            nc.sync.dma_start(cost[:], cos_d[:])
            nc.sync.dma_start(sint[:], sin_d[:])

            # Phase A: qT then c_kvT, one projection at a time (32KB weights)
            with (
                tc.tile_pool(name="wA", bufs=1) as p_wA,
                tc.tile_pool(name="xts", bufs=3) as p_xt,
                tc.tile_pool(name="psA", bufs=8, space="PSUM") as ps_a,
            ):
                for proj in range(2):  # 0 = q heads, 1 = c_kv
                    wp = p_wA.tile([128, 16, 512], F32R, tag="wA", name=f"wA{proj}")
                    for k in range(16):
                        nc.sync.dma_start(
                            wp[:, k, :], wqct_d[:, k, proj * 512 : (proj + 1) * 512]
                        )
                    for half in range(2):  # s-half of 1024
                        accs = []
                        for m in range(4):
                            for n2 in range(2):
                                accs.append(
                                    ps_a.tile(
                                        [128, 512], F32, tag="pa", name=f"pa{m}_{n2}"
                                    )
                                )
                        for k in range(16):
                            xt = p_xt.tile([128, 1024], F32R, tag="xt")
                            nc.sync.dma_start(
                                xt[:],
                                xt_d[
                                    k * 128 : (k + 1) * 128,
                                    half * 1024 : (half + 1) * 1024,
                                ],
                            )
                            for m in range(4):
                                for n2 in range(2):
                                    nc.tensor.matmul(
                                        accs[m * 2 + n2][:],
                                        wp[:, k, m * 128 : (m + 1) * 128],
                                        xt[:, n2 * 512 : (n2 + 1) * 512],
                                        start=(k == 0),
                                        stop=(k == 15),
                                    )
                        for m in range(4):
                            for n2 in range(2):
                                sq = slice(
                                    half * 1024 + n2 * 512, half * 1024 + n2 * 512 + 512
                                )
                                acc = accs[m * 2 + n2]
                                if proj == 0:
                                    _rope_evac(
                                        nc,
                                        p_tmp,
                                        acc[:],
                                        qt[:, m, sq],
                                        cost[:, sq],
                                        sint[:, sq],
                                    )
                                else:
                                    nc.scalar.copy(ckv[:, m, sq], acc[:])

            # Phase B: kT (+RoPE) and v from c_kvT
            with (
                tc.tile_pool(name="wB", bufs=1) as p_wB,
                tc.tile_pool(name="psB", bufs=4, space="PSUM") as ps_b,
                tc.tile_pool(name="psB2", bufs=4, space="PSUM") as ps_b2,
            ):
                wkvt = p_wB.tile([128, 4, 1024], F32R, tag="wkvt")
                for i in range(4):
                    nc.sync.dma_start(wkvt[:, i, :], wkvt_d[:, i, :])

                for m in range(4):  # dh chunk = head
                    for n in range(4):  # s-tile
                        sn = slice(n * 512, (n + 1) * 512)
                        acc = ps_b.tile([128, 512], F32, tag="pk")
                        for i in range(4):
                            nc.tensor.matmul(
                                acc[:],
                                wkvt[:, i, m * 128 : (m + 1) * 128],
                                ckv[:, i, sn],
                                start=(i == 0),
                                stop=(i == 3),
                            )
                        _rope_evac(
                            nc, p_tmp, acc[:], kt[:, m, sn], cost[:, sn], sint[:, sn]
                        )
                for m in range(16):  # s-chunk
                    acc = ps_b2.tile([128, 512], F32, tag="pv")
                    for i in range(4):
                        nc.tensor.matmul(
                            acc[:],
                            ckv[:, i, m * 128 : (m + 1) * 128],
                            wkvt[:, i, 512:1024],
                            start=(i == 0),
                            stop=(i == 3),
                        )
                    nc.scalar.copy(v[:, m, :], acc[:])

        # ============ Phase C: attention (scoresT/softmax/PV) ============
        # WoT preload happens in an outer scope so phase D starts immediately.
        with tc.tile_pool(name="wD", bufs=1) as p_wD:
            wot = p_wD.tile([128, 4, 2048], F32R, tag="wot")
            for i in range(4):
                nc.sync.dma_start(wot[:, i, :], wot_d[:, i, :])
            _phase_cd(
                nc, tc, qt, kt, v, ones_col, ones_row, mask_d, wot, out_d
            )


def _phase_cd(nc, tc, qt, kt, v, ones_col, ones_row, mask_d, wot, out_d):
    if True:
        with (
            tc.tile_pool(name="attn", bufs=1) as p_at,
            tc.tile_pool(name="ex", bufs=3) as p_ex,
            tc.tile_pool(name="bc", bufs=2) as p_bc,
            tc.tile_pool(name="psS", bufs=2, space="PSUM") as ps_s,
            tc.tile_pool(name="psO", bufs=2, space="PSUM") as ps_o,
            tc.tile_pool(name="psR", bufs=1, space="PSUM") as ps_r,
            tc.tile_pool(name="psB3", bufs=1, space="PSUM") as ps_bc,
        ):
            masks = p_at.tile([128, 4, 512], F32, tag="masks")
            nc.sync.dma_start(masks[:], mask_d[:])

            for h in range(HL):
                for t in range(4):  # sq tile of 512
                    sq = slice(t * 512, (t + 1) * 512)
                    nj = 4 * (t + 1)  # causal sk chunks
                    acc_o = ps_o.tile([128, 512], F32, tag="o")
                    acc_s = ps_r.tile([1, 512], F32, tag="sum")

                    # software-pipelined: scores/exp of pair p+1 are emitted
                    # before sums/pv of pair p so PE never waits on ACT.
                    stages = []
                    for jj in range(0, nj, 2):
                        stages.append(jj)
                    pend = None
                    for jj in stages + [None]:
                        if jj is not None:
                            ps = ps_s.tile([128, 1024], F32, tag="s")
                            for d_ in range(2):
                                j = jj + d_
                                nc.tensor.matmul(
                                    ps[:, d_ * 512 : (d_ + 1) * 512],
                                    kt[:, h, j * 128 : (j + 1) * 128],
                                    qt[:, h, sq],
                                    start=True,
                                    stop=True,
                                )
                            ex = p_ex.tile([128, 1024], F32R, tag="ex")
                            nc.scalar.activation(ex[:], ps[:], EXP, scale=SCALE)
                            for d_ in range(2):
                                j = jj + d_
                                di = j - 4 * t
                                if di >= 0:  # diagonal chunk -> causal mask
                                    exs = ex[:, d_ * 512 : (d_ + 1) * 512]
                                    nc.vector.tensor_mul(exs, exs, masks[:, di, :])
                            cur = (jj, ex)
                        else:
                            cur = None
                        if pend is not None:
                            pjj, pex = pend
                            for d_ in range(2):
                                j = pjj + d_
                                exs = pex[:, d_ * 512 : (d_ + 1) * 512]
                                nc.tensor.matmul(
                                    acc_s[:],
                                    ones_col[:],
                                    exs,
                                    start=(j == 0),
                                    stop=(j == nj - 1),
                                )
                                nc.tensor.matmul(
                                    acc_o[:],
                                    v[:, j, h * 128 : (h + 1) * 128],
                                    exs,
                                    start=(j == 0),
                                    stop=(j == nj - 1),
                                )
                        pend = cur

                    # normalize: oT = acc_o * bcast(1/sums)
                    srow = p_bc.tile([1, 512], F32R, tag="srow")
                    nc.scalar.copy(srow[:], acc_s[:])
                    pb = ps_bc.tile([128, 512], F32, tag="b")
                    nc.tensor.matmul(pb[:], ones_row[:], srow[:], start=True, stop=True)
                    bc = p_bc.tile([128, 512], F32, tag="bc")
                    nc.vector.reciprocal(bc[:], pb[:])
                    # oT overwrites qT storage (qT[h, sq] fully consumed)
                    nc.vector.tensor_mul(qt[:, h, sq], acc_o[:], bc[:])

    # ================= Phase D: output projection =================
    with (
        tc.tile_pool(name="oev", bufs=4) as p_oev,
        tc.tile_pool(name="psD", bufs=4, space="PSUM") as ps_d,
    ):
        for m in range(16):  # dout chunk
            for n in range(4):  # s tile
                sn = slice(n * 512, (n + 1) * 512)
                acc = ps_d.tile([128, 512], F32, tag="pd")
                for i in range(4):
                    nc.tensor.matmul(
                        acc[:],
                        wot[:, i, m * 128 : (m + 1) * 128],
                        qt[:, i, sn],
                        start=(i == 0),
                        stop=(i == 3),
                    )
                o = p_oev.tile([128, 512], F32, tag="oev")
                nc.scalar.copy(o[:], acc[:])
                nc.sync.dma_start(out_d[m * 128 : (m + 1) * 128, sn], o[:])


def _rope_tables():
    inv = 1.0 / (10000.0 ** (np.arange(0, DH, 2, dtype=np.float32) / DH))
    t = np.arange(S, dtype=np.float32)
    freqs = np.outer(t, inv)  # (S, 64)
    emb = np.concatenate([freqs, freqs], axis=-1)  # (S, 128)
    return (
        np.ascontiguousarray(np.cos(emb).T.astype(np.float32)),
        np.ascontiguousarray(np.sin(emb).T.astype(np.float32)),
    )


def _masks():
    p = np.arange(128)[:, None]
    f = np.arange(512)[None, :]
    m = np.zeros((128, 4, 512), np.float32)
    for i in range(4):
        m[:, i, :] = (p <= f - 128 * i).astype(np.float32)
    return m


def kernel(hidden_states, Wq, Wc, Wk, Wv, Wo, _trace=False):
    hidden_states = np.asarray(hidden_states, dtype=np.float32)
    Wq = np.asarray(Wq, dtype=np.float32)
    Wc = np.asarray(Wc, dtype=np.float32)
    Wk = np.asarray(Wk, dtype=np.float32)
    Wv = np.asarray(Wv, dtype=np.float32)
    Wo = np.asarray(Wo, dtype=np.float32)

    if "nc" not in _CACHED:
        _CACHED["nc"] = _build()
    nc = _CACHED["nc"]

    cost, sint = _rope_tables()
    masks = _masks()
    ones = np.ones((128, 128), np.float32)
    wct = np.ascontiguousarray(Wc.T)  # (D, DL)
    xts = [np.ascontiguousarray(hidden_states[b].T) for b in range(B)]

    in_maps = []
    for c in range(NCORES):
        b, hg = divmod(c, HL)
        hs = hg * HF
        wq = Wq[hs : hs + HF].T  # (D, HF)
        wqct = np.concatenate([wq, wct], axis=1)  # (D, 1024)
        wqct = np.ascontiguousarray(wqct.reshape(16, 128, 1024).transpose(1, 0, 2))
        wk = Wk[hs : hs + HF].T  # (DL, HF)
        wv = Wv[hs : hs + HF].T
        wkvt = np.concatenate([wk, wv], axis=1)  # (DL, 1024)
        wkvt = np.ascontiguousarray(wkvt.reshape(4, 128, 1024).transpose(1, 0, 2))
        wot = Wo[:, hs : hs + HF].T  # (HF, D)
        wot = np.ascontiguousarray(wot.reshape(4, 128, 2048).transpose(1, 0, 2))
        in_maps.append(
            {
                "xt": xts[b],
                "wqct": wqct,
                "wkvt": wkvt,
                "wot": wot,
                "cost": cost,
                "sint": sint,
                "masks": masks,
                "ones": ones,
            }
        )

    res = run_bass_kernel_spmd(
        nc, in_maps, core_ids=list(range(NCORES)), trace=_trace
    )
    _CACHED["last_results"] = res

    out = np.empty((B, S, D), np.float32)
    for b in range(B):
        acc = res.results[4 * b]["outt"].astype(np.float32)
        for c in range(4 * b + 1, 4 * b + 4):
            acc = acc + res.results[c]["outt"]
        out[b] = acc.T
    return out


# revision 6
# speedup vs baseline: 1.0709x; 1.0709x over previous
"""MLA attention kernel for Trainium2 (8 NeuronCores, Bass/Tile).

Problem (nn_MLAAttention): B=2, S=2048, D=2048, 16 heads x 128, latent 512,
RoPE, causal softmax, output projection.

Sharding: core c handles batch b=c//4 and 4 heads hg=c%4 (tensor parallel over
heads, data parallel over batch). Each core computes a partial output
(attn-out of its heads through its Wo column-slice); the host sums partials
within a batch and transposes back.

On-core dataflow (all activations kept feature-major, "transposed"):
  XT[d,s] --Wq/Wc--> qT[dh,s] (+RoPE), c_kvT[dl,s]
  c_kvT --Wk--> kT[dh,s] (+RoPE);  c_kvT --Wv--> v[s,hf] (natural)
  scoresT[sk,sq] = kT.T-slice @ qT-slice  (per head, causal tiles only)
  expT = exp(scores/sqrt(dh)) * causal_mask
  sums[1,sq] = ones.T @ expT (partition reduce via PE)
  oT~[dh,sq] = v-slice.T @ expT  (unnormalized)
  oT = oT~ * bcast(1/sums)   (PE K=1 broadcast matmul + DVE reciprocal/mul)
  outT[dout,s] = WoT.T-slices @ oT  (partial output projection)

All matmuls run as float32r (full rate, ~1e-4 rounding vs fp32).
"""

import sys

if "/opt/trn_rl_repo" not in sys.path:
    sys.path.insert(0, "/opt/trn_rl_repo")

import math
import numpy as np

import concourse.bass as bass
import concourse.tile as tile
from concourse import bacc, mybir
from concourse.bass_utils import run_bass_kernel_spmd

F32 = mybir.dt.float32
F32R = mybir.dt.float32r
EXP = mybir.ActivationFunctionType.Exp

B, S, D, DL, H, DH = 2, 2048, 2048, 512, 16, 128
HL = 4  # heads per core
HF = HL * DH  # 512 local head-features
NCORES = 8
SCALE = 1.0 / math.sqrt(DH)

_CACHED = {}


def _build():
    nc = bacc.Bacc("TRN2", target_bir_lowering=False, debug=False)

    xt_d = nc.dram_tensor("xt", [D, S], F32R, kind="ExternalInput")
    wqct_d = nc.dram_tensor("wqct", [128, 16, 1024], F32R, kind="ExternalInput")
    wkvt_d = nc.dram_tensor("wkvt", [128, 4, 1024], F32R, kind="ExternalInput")
    wot_d = nc.dram_tensor("wot", [128, 4, 2048], F32R, kind="ExternalInput")
    cos_d = nc.dram_tensor("cost", [128, S], F32, kind="ExternalInput")
    sin_d = nc.dram_tensor("sint", [128, S], F32, kind="ExternalInput")
    mask_d = nc.dram_tensor("masks", [128, 4, 512], F32, kind="ExternalInput")
    ones_d = nc.dram_tensor("ones", [128, 128], F32R, kind="ExternalInput")
    out_d = nc.dram_tensor("outt", [D, S], F32, kind="ExternalOutput")

    with tile.TileContext(nc) as tc:
        _body(nc, tc, xt_d, wqct_d, wkvt_d, wot_d, cos_d, sin_d, mask_d, ones_d, out_d)
    nc.compile()
    return nc


def _rope_evac(nc, tmp_pool, psum, out_ap, cos_ap, sin_ap):
    """out = psum*cos + rotate_half(psum)*sin, written as float32r.

    psum: [128, n] PSUM (dh on partitions); cos/sin: [128, n] SBUF slices.
    """
    n = psum.shape[-1]
    t1 = tmp_pool.tile([128, 512], F32, tag="rope1")
    nc.vector.tensor_mul(t1[:, :n], psum[:], cos_ap)
    t2 = tmp_pool.tile([128, 512], F32, tag="rope2")
    nc.vector.tensor_mul(t2[0:64, :n], psum[64:128, :], sin_ap[0:64, :])
    nc.vector.tensor_mul(t2[64:128, :n], psum[0:64, :], sin_ap[64:128, :])
    nc.vector.tensor_sub(out_ap[0:64, :], t1[0:64, :n], t2[0:64, :n])
    nc.vector.tensor_add(out_ap[64:128, :], t1[64:128, :n], t2[64:128, :n])


def _body(nc, tc, xt_d, wqct_d, wkvt_d, wot_d, cos_d, sin_d, mask_d, ones_d, out_d):
    import contextlib

    with contextlib.ExitStack() as ctx:
        ent = ctx.enter_context

        # ---- persistent pools (right heap side; phase pools use left) ----
        p_qt = ent(tc.tile_pool(name="qt", bufs=1, side="right"))
        p_small = ent(tc.tile_pool(name="small", bufs=1, side="right"))

        qt = p_qt.tile([128, HL, S], F32R, tag="qt")  # [dh, h, s]
        ones_col = p_small.tile([128, 1], F32R, tag="ones_col")
        ones_row = p_small.tile([1, 128], F32R, tag="ones_row")

        nc.sync.dma_start(ones_col[:], ones_d[:, 0:1])
        nc.sync.dma_start(ones_row[:], ones_d[0:1, :])

        # ============ Phases A+B: projections (scoped pools) ============
        with (
            tc.tile_pool(name="ck", bufs=1) as p_ck,
            tc.tile_pool(name="tmp", bufs=3) as p_tmp,
        ):
            ckv = p_ck.tile([128, 4, S], F32R, tag="ckv")  # [dl%128, dl//128, s]
            cost = p_ck.tile([128, S], F32, tag="cos")
            sint = p_ck.tile([128, S], F32, tag="sin")
            nc.sync.dma_start(cost[:], cos_d[:])
            nc.sync.dma_start(sint[:], sin_d[:])

            # Phase A: fused qT + c_kvT projection, single pass over XT.
            # 8 PSUM banks = 4 c_kv chunks (ACT evac, fast) + 4 q heads
            # (DVE RoPE evac); c banks are allocated/issued first so the
            # next s-quarter's matmuls restart on ACT-freed banks.
            with (
                tc.tile_pool(name="wA", bufs=1) as p_wA,
                tc.tile_pool(name="xts", bufs=3) as p_xt,
                tc.tile_pool(name="psA", bufs=8, space="PSUM") as ps_a,
            ):
                wqct = p_wA.tile([128, 16, 1024], F32R, tag="wA")
                for k in range(16):
                    nc.sync.dma_start(wqct[:, k, :], wqct_d[:, k, :])
                for q in range(4):  # s-quarter
                    sq = slice(q * 512, (q + 1) * 512)
                    accs = [
                        ps_a.tile([128, 512], F32, tag="pa", name=f"pa{m}")
                        for m in range(8)
                    ]  # accs[0:4] = c_kv chunks, accs[4:8] = q heads
                    for k in range(16):
                        xt = p_xt.tile([128, 512], F32R, tag="xt")
                        nc.sync.dma_start(xt[:], xt_d[k * 128 : (k + 1) * 128, sq])
                        for i in range(8):
                            # i<4 -> c_kv (weight cols 512+), i>=4 -> q heads
                            wcol = 512 + i * 128 if i < 4 else (i - 4) * 128
                            nc.tensor.matmul(
                                accs[i][:],
                                wqct[:, k, wcol : wcol + 128],
                                xt[:],
                                start=(k == 0),
                                stop=(k == 15),
                            )
                    for i in range(4):
                        nc.scalar.copy(ckv[:, i, sq], accs[i][:])
                    for i in range(4):
                        _rope_evac(
                            nc,
                            p_tmp,
                            accs[4 + i][:],
                            qt[:, i, sq],
                            cost[:, sq],
                            sint[:, sq],
                        )

            # Phase B: kT (+RoPE) and v from c_kvT, interleaved so ACT
            # (v evac) and DVE (kT RoPE) work in parallel.
            p_kv = ent(tc.tile_pool(name="kv", bufs=1, side="right"))
            with (
                tc.tile_pool(name="wB", bufs=1) as p_wB,
                tc.tile_pool(name="psB", bufs=4, space="PSUM") as ps_b,
                tc.tile_pool(name="psB2", bufs=4, space="PSUM") as ps_b2,
            ):
                kt = p_kv.tile([128, HL, S], F32R, tag="kt")  # [dh, h, s]
                v = p_kv.tile([128, 16, HF], F32R, tag="v")  # [s%128, s//128, hf]
                wkvt = p_wB.tile([128, 4, 1024], F32R, tag="wkvt")
                for i in range(4):
                    nc.sync.dma_start(wkvt[:, i, :], wkvt_d[:, i, :])

                for g in range(16):
                    # kT group: head m = g//4, s-tile n = g%4
                    m, n = divmod(g, 4)
                    sn = slice(n * 512, (n + 1) * 512)
                    acc = ps_b.tile([128, 512], F32, tag="pk")
                    for i in range(4):
                        nc.tensor.matmul(
                            acc[:],
                            wkvt[:, i, m * 128 : (m + 1) * 128],
                            ckv[:, i, sn],
                            start=(i == 0),
                            stop=(i == 3),
                        )
                    _rope_evac(
                        nc, p_tmp, acc[:], kt[:, m, sn], cost[:, sn], sint[:, sn]
                    )
                    # v group: s-chunk g
                    acc2 = ps_b2.tile([128, 512], F32, tag="pv")
                    for i in range(4):
                        nc.tensor.matmul(
                            acc2[:],
                            ckv[:, i, g * 128 : (g + 1) * 128],
                            wkvt[:, i, 512:1024],
                            start=(i == 0),
                            stop=(i == 3),
                        )
                    nc.scalar.copy(v[:, g, :], acc2[:])

        # ============ Phase C: attention (scoresT/softmax/PV) ============
        # WoT preload happens in an outer scope so phase D starts immediately.
        with tc.tile_pool(name="wD", bufs=1, side="right") as p_wD:
            wot = p_wD.tile([128, 4, 2048], F32R, tag="wot")
            for i in range(4):
                nc.sync.dma_start(wot[:, i, :], wot_d[:, i, :])
            _phase_cd(
                nc, tc, qt, kt, v, ones_col, ones_row, mask_d, wot, out_d
            )


def _phase_cd(nc, tc, qt, kt, v, ones_col, ones_row, mask_d, wot, out_d):
    if True:
        with (
            tc.tile_pool(name="attn", bufs=1) as p_at,
            tc.tile_pool(name="ex", bufs=3) as p_ex,
            tc.tile_pool(name="bc", bufs=2) as p_bc,
            tc.tile_pool(name="psS", bufs=2, space="PSUM") as ps_s,
            tc.tile_pool(name="psO", bufs=2, space="PSUM") as ps_o,
            tc.tile_pool(name="psR", bufs=1, space="PSUM") as ps_r,
            tc.tile_pool(name="psB3", bufs=1, space="PSUM") as ps_bc,
        ):
            masks = p_at.tile([128, 4, 512], F32, tag="masks")
            nc.sync.dma_start(masks[:], mask_d[:])

            for h in range(HL):
                for t in range(4):  # sq tile of 512
                    sq = slice(t * 512, (t + 1) * 512)
                    nj = 4 * (t + 1)  # causal sk chunks
                    acc_o = ps_o.tile([128, 512], F32, tag="o")
                    acc_s = ps_r.tile([1, 512], F32, tag="sum")

                    # software-pipelined: scores/exp of pair p+1 are emitted
                    # before sums/pv of pair p so PE never waits on ACT.
                    stages = []
                    for jj in range(0, nj, 2):
                        stages.append(jj)
                    pend = None
                    for jj in stages + [None]:
                        if jj is not None:
                            ps = ps_s.tile([128, 1024], F32, tag="s")
                            for d_ in range(2):
                                j = jj + d_
                                nc.tensor.matmul(
                                    ps[:, d_ * 512 : (d_ + 1) * 512],
                                    kt[:, h, j * 128 : (j + 1) * 128],
                                    qt[:, h, sq],
                                    start=True,
                                    stop=True,
                                )
                            ex = p_ex.tile([128, 1024], F32R, tag="ex")
                            nc.scalar.activation(ex[:], ps[:], EXP, scale=SCALE)
                            for d_ in range(2):
                                j = jj + d_
                                di = j - 4 * t
                                if di >= 0:  # diagonal chunk -> causal mask
                                    exs = ex[:, d_ * 512 : (d_ + 1) * 512]
                                    nc.vector.tensor_mul(exs, exs, masks[:, di, :])
                            cur = (jj, ex)
                        else:
                            cur = None
                        if pend is not None:
                            pjj, pex = pend
                            for d_ in range(2):
                                j = pjj + d_
                                exs = pex[:, d_ * 512 : (d_ + 1) * 512]
                                nc.tensor.matmul(
                                    acc_s[:],
                                    ones_col[:],
                                    exs,
                                    start=(j == 0),
                                    stop=(j == nj - 1),
                                )
                                nc.tensor.matmul(
                                    acc_o[:],
                                    v[:, j, h * 128 : (h + 1) * 128],
                                    exs,
                                    start=(j == 0),
                                    stop=(j == nj - 1),
                                )
                        pend = cur

                    # normalize: oT = acc_o * bcast(1/sums)
                    srow = p_bc.tile([1, 512], F32R, tag="srow")
                    nc.scalar.copy(srow[:], acc_s[:])
                    pb = ps_bc.tile([128, 512], F32, tag="b")
                    nc.tensor.matmul(pb[:], ones_row[:], srow[:], start=True, stop=True)
                    bc = p_bc.tile([128, 512], F32, tag="bc")
                    rscr = p_bc.tile([128, 512], F32, tag="rscr")
                    nc.vector.reciprocal_approx_accurate(bc[:], pb[:], rscr[:])
                    # oT overwrites qT storage (qT[h, sq] fully consumed)
                    nc.vector.tensor_mul(qt[:, h, sq], acc_o[:], bc[:])

    # ================= Phase D: output projection =================
    with (
        tc.tile_pool(name="oev", bufs=4) as p_oev,
        tc.tile_pool(name="psD", bufs=4, space="PSUM") as ps_d,
    ):
        for m in range(16):  # dout chunk
            for n in range(4):  # s tile
                sn = slice(n * 512, (n + 1) * 512)
                acc = ps_d.tile([128, 512], F32, tag="pd")
                for i in range(4):
                    nc.tensor.matmul(
                        acc[:],
                        wot[:, i, m * 128 : (m + 1) * 128],
                        qt[:, i, sn],
                        start=(i == 0),
                        stop=(i == 3),
                    )
                o = p_oev.tile([128, 512], F32, tag="oev")
                nc.scalar.copy(o[:], acc[:])
                nc.sync.dma_start(out_d[m * 128 : (m + 1) * 128, sn], o[:])


def _rope_tables():
    inv = 1.0 / (10000.0 ** (np.arange(0, DH, 2, dtype=np.float32) / DH))
    t = np.arange(S, dtype=np.float32)
    freqs = np.outer(t, inv)  # (S, 64)
    emb = np.concatenate([freqs, freqs], axis=-1)  # (S, 128)
    return (
        np.ascontiguousarray(np.cos(emb).T.astype(np.float32)),
        np.ascontiguousarray(np.sin(emb).T.astype(np.float32)),
    )


def _masks():
    p = np.arange(128)[:, None]
    f = np.arange(512)[None, :]
    m = np.zeros((128, 4, 512), np.float32)
    for i in range(4):
        m[:, i, :] = (p <= f - 128 * i).astype(np.float32)
    return m


def kernel(hidden_states, Wq, Wc, Wk, Wv, Wo, _trace=False):
    hidden_states = np.asarray(hidden_states, dtype=np.float32)
    Wq = np.asarray(Wq, dtype=np.float32)
    Wc = np.asarray(Wc, dtype=np.float32)
    Wk = np.asarray(Wk, dtype=np.float32)
    Wv = np.asarray(Wv, dtype=np.float32)
    Wo = np.asarray(Wo, dtype=np.float32)

    if "nc" not in _CACHED:
        _CACHED["nc"] = _build()
    nc = _CACHED["nc"]

    cost, sint = _rope_tables()
    masks = _masks()
    ones = np.ones((128, 128), np.float32)
    wct = np.ascontiguousarray(Wc.T)  # (D, DL)
    xts = [np.ascontiguousarray(hidden_states[b].T) for b in range(B)]

    in_maps = []
    for c in range(NCORES):
        b, hg = divmod(c, HL)
        hs = hg * HF
        wq = Wq[hs : hs + HF].T  # (D, HF)
        wqct = np.concatenate([wq, wct], axis=1)  # (D, 1024)
        wqct = np.ascontiguousarray(wqct.reshape(16, 128, 1024).transpose(1, 0, 2))
        wk = Wk[hs : hs + HF].T  # (DL, HF)
        wv = Wv[hs : hs + HF].T
        wkvt = np.concatenate([wk, wv], axis=1)  # (DL, 1024)
        wkvt = np.ascontiguousarray(wkvt.reshape(4, 128, 1024).transpose(1, 0, 2))
        wot = Wo[:, hs : hs + HF].T  # (HF, D)
        wot = np.ascontiguousarray(wot.reshape(4, 128, 2048).transpose(1, 0, 2))
        in_maps.append(
            {
                "xt": xts[b],
                "wqct": wqct,
                "wkvt": wkvt,
                "wot": wot,
                "cost": cost,
                "sint": sint,
                "masks": masks,
                "ones": ones,
            }
        )

    res = run_bass_kernel_spmd(
        nc, in_maps, core_ids=list(range(NCORES)), trace=_trace
    )
    _CACHED["last_results"] = res

    out = np.empty((B, S, D), np.float32)
    for b in range(B):
        acc = res.results[4 * b]["outt"].astype(np.float32)
        for c in range(4 * b + 1, 4 * b + 4):
            acc = acc + res.results[c]["outt"]
        out[b] = acc.T
    return out
